# revision 1
# baseline (speedup 1.0000x reference)
import sys

sys.path.insert(0, "/opt/trn_rl_repo")
import numpy as np

DIM = 1024
HEADS = 16
HEAD_DIM = 64
HID = 4096
EPS = 1e-5
NQ = 512          # queries per core
NK = 2048
P = 128
KC = DIM // P     # 8 contraction chunks
NQT = NQ // P     # 4 query tiles
NKT = NK // P     # 16 kv chunks

_CACHE = {}


def _build():
    import concourse.bacc as bacc
    import concourse.tile as tile
    from concourse import mybir
    from concourse.masks import make_identity
    from contextlib import ExitStack

    F32 = mybir.dt.float32
    F32R = mybir.dt.float32r
    AF = mybir.ActivationFunctionType

    nc = bacc.Bacc(None, target_bir_lowering=False, debug=False)

    tgt = nc.declare_dram_parameter("tgt", [NQ, DIM], F32, isOutput=False)
    emb = nc.declare_dram_parameter("emb", [DIM, NK], F32R, isOutput=False)
    bv = nc.declare_dram_parameter("bv", [DIM], F32, isOutput=False)
    # weights: wq/wo as row tiles; wk/wv pretiled by (kc, quarter); w1 by (kc, grp)
    wq = nc.declare_dram_parameter("wq", [DIM, DIM], F32R, isOutput=False)
    wk = nc.declare_dram_parameter("wk", [KC, 4, P, 256], F32R, isOutput=False)
    wv = nc.declare_dram_parameter("wv", [KC, 4, P, 256], F32R, isOutput=False)
    wo = nc.declare_dram_parameter("wo", [DIM, DIM], F32R, isOutput=False)
    w1 = nc.declare_dram_parameter("w1", [KC, 4, P, 1024], F32R, isOutput=False)
    w2 = nc.declare_dram_parameter("w2", [2, HID // P, P, 512], F32R, isOutput=False)
    # bias pack: [128, 64] = bq(8) | bk(8) | bo(8) | b2(8) | b1(32)
    bias_pack = nc.declare_dram_parameter("bias_pack", [P, 64], F32, isOutput=False)
    out = nc.declare_dram_parameter("out", [NQ, DIM], F32, isOutput=True)

    def bcast_ap(vec, n):
        import concourse.bass as bass
        return bass.AP(tensor=vec.tensor, offset=vec.offset, ap=[[0, P], [1, n]])

    with tile.TileContext(nc) as tc, ExitStack() as S:
        const = S.enter_context(tc.tile_pool(name="const", bufs=1))

        ident = const.tile([P, P], F32)
        make_identity(nc, ident)
        identr = const.tile([P, P], F32R)
        nc.scalar.activation(identr[:], ident[:], AF.Copy)
        ones_f = const.tile([P, 64], F32)
        nc.vector.memset(ones_f[:], 1.0)
        eps_t = const.tile([P, 1], F32)
        nc.vector.memset(eps_t[:], EPS)

        bv_b = const.tile([P, DIM], F32)
        nc.gpsimd.dma_start(out=bv_b[:], in_=bcast_ap(bv[:], DIM))

        bp = const.tile([P, 64], F32)
        nc.sync.dma_start(out=bp[:], in_=bias_pack[:, :])
        bq_s = bp[:, 0:8]
        bk_s = bp[:, 8:16]
        bo_s = bp[:, 16:24]
        b2_s = bp[:, 24:32]
        b1_s = bp[:, 32:64]

        def layer_norm_tiles(src_tiles, dst_tiles, pool):
            for t in range(len(src_tiles)):
                x = src_tiles[t]
                st = pool.tile([P, 2, nc.vector.BN_STATS_DIM], F32, name=f"st{t}", tag="st")
                for sg in range(2):
                    nc.vector.bn_stats(out=st[:, sg, :], in_=x[:, sg * 512:(sg + 1) * 512])
                mv = pool.tile([P, nc.vector.BN_AGGR_DIM], F32, name=f"mv{t}", tag="mv")
                nc.vector.bn_aggr(out=mv[:], in_=st[:])
                rstd = pool.tile([P, 1], F32, name=f"rstd{t}", tag="rstd")
                nc.scalar.activation(out=rstd[:], in_=mv[:, 1:2], func=AF.Sqrt,
                                     bias=eps_t[:], scale=1.0)
                nc.vector.reciprocal(out=rstd[:], in_=rstd[:])
                y = dst_tiles[t]
                nc.vector.tensor_scalar(out=y[:], in0=x[:], scalar1=mv[:, 0:1],
                                        scalar2=rstd[:], op0=mybir.AluOpType.subtract,
                                        op1=mybir.AluOpType.mult)

        att = S.enter_context(tc.tile_pool(name="att", bufs=1))  # qT, ctxT (live to out-proj)
        qT = [att.tile([P, NQ], F32R, name=f"qT{m}") for m in range(KC)]
        ctxT = [att.tile([P, NQ], F32R, name=f"ctxT{m}") for m in range(KC)]

        embT_cm = tc.tile_pool(name="embT", bufs=1)
        embT = embT_cm.__enter__()
        eT = [embT.tile([P, NK], F32R, name=f"eT{k}") for k in range(KC)]

        # ---------- Phase 1: LN(tgt) -> lnT; emb -> embT; qT ----------
        with tc.tile_pool(name="lnq", bufs=1) as lnq, \
             tc.tile_pool(name="lnw", bufs=4) as lnw, \
             tc.tile_pool(name="tp_ps", bufs=4, space="PSUM") as tp_ps:
            for nt4 in range(4):
                for k in range(KC):
                    nc.sync.dma_start(out=eT[k][:, nt4 * 512:(nt4 + 1) * 512],
                                      in_=emb[k * P:(k + 1) * P, nt4 * 512:(nt4 + 1) * 512])
            ln_tiles = [lnq.tile([P, DIM], F32, name=f"ln{t}") for t in range(NQT)]
            for t in range(NQT):
                nc.sync.dma_start(out=ln_tiles[t][:], in_=tgt[t * P:(t + 1) * P, :])
            layer_norm_tiles(ln_tiles, ln_tiles, lnw)
            lnT = [lnq.tile([P, NQ], F32R, name=f"lnT{k}") for k in range(KC)]
            for t in range(NQT):
                for k in range(KC):
                    pt = tp_ps.tile([P, P], F32, name="pt", tag="tp")
                    nc.tensor.transpose(pt[:], ln_tiles[t][:, k * P:(k + 1) * P], ident[:])
                    if k % 2 == 0:
                        nc.vector.tensor_copy(lnT[k][:, t * P:(t + 1) * P], pt[:])
                    else:
                        nc.scalar.activation(lnT[k][:, t * P:(t + 1) * P], pt[:], AF.Copy)

            with tc.tile_pool(name="wqp", bufs=1) as wqp, \
                 tc.tile_pool(name="proj_ps", bufs=4, space="PSUM") as proj_ps:
                wq_sb = [wqp.tile([P, DIM], F32R, name=f"wq{k}") for k in range(KC)]
                for k in range(KC):
                    nc.sync.dma_start(out=wq_sb[k][:], in_=wq[k * P:(k + 1) * P, :])
                for m in range(KC):
                    ps = proj_ps.tile([P, NQ], F32, name="qps", tag="proj")
                    for k in range(KC):
                        nc.tensor.matmul(ps[:], wq_sb[k][:, m * P:(m + 1) * P], lnT[k][:],
                                         start=(k == 0), stop=(k == KC - 1))
                    nc.vector.tensor_scalar_add(qT[m][:], ps[:], bq_s[:, m:m + 1])

        # ---------- Phase 2: per quarter (4 heads): kT, v, attention ----------
        with tc.tile_pool(name="kvw", bufs=2) as kvw, \
             tc.tile_pool(name="kvt_p", bufs=2) as kvt_p, \
             tc.tile_pool(name="vq_p", bufs=2) as vq_p, \
             tc.tile_pool(name="ax", bufs=4) as ax, \
             tc.tile_pool(name="ax2", bufs=2) as ax2, \
             tc.tile_pool(name="ax3", bufs=1) as ax3, \
             tc.tile_pool(name="gps", bufs=2, space="PSUM") as gps, \
             tc.tile_pool(name="scp", bufs=4, space="PSUM") as scp, \
             tc.tile_pool(name="cxp", bufs=2, space="PSUM") as cxp:
            for q in range(4):          # quarter = 2 pairs = 4 heads
                wk_q = [kvw.tile([P, 256], F32R, name=f"wkq{q}_{k}", tag=f"wk{k}")
                        for k in range(KC)]
                wv_q = [kvw.tile([P, 256], F32R, name=f"wvq{q}_{k}", tag=f"wv{k}")
                        for k in range(KC)]
                for k in range(KC):
                    nc.sync.dma_start(out=wk_q[k][:], in_=wk[k, q])
                    nc.sync.dma_start(out=wv_q[k][:], in_=wv[k, q])

                v_q = [vq_p.tile([P, 4, 66], F32R, name=f"vq{q}_{kvt}", tag=f"v{kvt}")
                       for kvt in range(NKT)]
                bv_view = bv_b[:, q * 256:(q + 1) * 256].rearrange("p (h d) -> p h d", d=64)
                for kvt in range(NKT):
                    ps = gps.tile([P, 256], F32, name="vps", tag="gp")
                    for k in range(KC):
                        nc.tensor.matmul(ps[:], eT[k][:, kvt * P:(kvt + 1) * P], wv_q[k][:],
                                         start=(k == 0), stop=(k == KC - 1))
                    nc.vector.tensor_copy(v_q[kvt][:, :, 1:65],
                                          ps[:].rearrange("p (h d) -> p h d", d=64))
                    nc.vector.tensor_add(v_q[kvt][:, :, 1:65], v_q[kvt][:, :, 1:65], bv_view)
                    nc.vector.tensor_copy(v_q[kvt][:, :, 0], ones_f[:, 0:4])
                    nc.vector.tensor_copy(v_q[kvt][:, :, 65], ones_f[:, 0:4])

                for i2 in range(2):
                    pr = q * 2 + i2
                    kT = kvt_p.tile([P, NK], F32R, name=f"kT{pr}", tag="kT")
                    for nt in range(4):
                        ps = gps.tile([P, 512], F32, name="kps", tag="gp")
                        for k in range(KC):
                            nc.tensor.matmul(ps[:], wk_q[k][:, i2 * P:(i2 + 1) * P],
                                             eT[k][:, nt * 512:(nt + 1) * 512],
                                             start=(k == 0), stop=(k == KC - 1))
                        nc.vector.tensor_scalar_add(kT[:, nt * 512:(nt + 1) * 512], ps[:],
                                                    bk_s[:, pr:pr + 1])

                    for hl in range(2):
                        hq = i2 * 2 + hl          # head index within quarter
                        cps = cxp.tile([P, NQ], F32, name="cps", tag="ctx")
                        for kvt in range(NKT):
                            sc = scp.tile([P, NQ], F32, name="sc", tag="sc")
                            nc.tensor.matmul(sc[:], kT[hl * 64:(hl + 1) * 64, kvt * P:(kvt + 1) * P],
                                             qT[pr][hl * 64:(hl + 1) * 64, :], start=True, stop=True)
                            ex = ax.tile([P, NQ], F32R, name="ex", tag="ex")
                            nc.scalar.activation(ex[:], sc[:], AF.Exp, scale=0.125)
                            nc.tensor.matmul(cps[0:65, :], v_q[kvt][:, hq, 1:66], ex[:],
                                             start=(kvt == 0), stop=(kvt == NKT - 1))
                        rl = ax3.tile([P, NQ], F32, name="rl", tag="rl")
                        nc.vector.reciprocal(out=rl[64:65, :], in_=cps[64:65, :])
                        rl0 = ax3.tile([1, NQ], F32, name="rl0", tag="rl0")
                        nc.sync.dma_start(out=rl0[0:1, :], in_=rl[64:65, :])
                        bcs = ax2.tile([64, NQ], F32, name="bcs", tag="bcs")
                        nc.gpsimd.partition_broadcast(bcs[:], rl0[0:1, :], channels=64)
                        if hl == 0:
                            nc.vector.tensor_mul(ctxT[pr][0:64, :], cps[0:64, :], bcs[:])
                        else:
                            ctmp = ax3.tile([64, NQ], F32R, name="ctmp", tag="ctmp")
                            nc.vector.tensor_mul(ctmp[:], cps[0:64, :], bcs[:])
                            nc.sync.dma_start(out=ctxT[pr][64:128, :], in_=ctmp[:])

        embT_cm.__exit__(None, None, None)

        # ---------- Phase 3: out-proj, +tgt residual, LN2 ----------
        outp = S.enter_context(tc.tile_pool(name="outp", bufs=1))
        tgt2 = [outp.tile([P, DIM], F32, name=f"tgt2_{t}") for t in range(NQT)]

        with tc.tile_pool(name="wop", bufs=1) as wop, \
             tc.tile_pool(name="oy", bufs=4) as oy, \
             tc.tile_pool(name="o_ps", bufs=4, space="PSUM") as o_ps, \
             tc.tile_pool(name="ot_ps", bufs=4, space="PSUM") as ot_ps:
            wo_sb = [wop.tile([P, DIM], F32R, name=f"wo{k}") for k in range(KC)]
            for k in range(KC):
                nc.sync.dma_start(out=wo_sb[k][:], in_=wo[k * P:(k + 1) * P, :])
            tgt_r = [wop.tile([P, DIM], F32, name=f"tgtr{t}") for t in range(NQT)]
            for t in range(NQT):
                nc.sync.dma_start(out=tgt_r[t][:], in_=tgt[t * P:(t + 1) * P, :])
            st2 = [outp.tile([P, 2, nc.vector.BN_STATS_DIM], F32, name=f"st2_{t}")
                   for t in range(NQT)]
            for mcg in range(2):
                for mc4 in range(4):
                    mc = mcg * 4 + mc4
                    ps = o_ps.tile([P, NQ], F32, name="ops", tag="op")
                    for prr in range(KC):
                        nc.tensor.matmul(ps[:], wo_sb[prr][:, mc * P:(mc + 1) * P], ctxT[prr][:],
                                         start=(prr == 0), stop=(prr == KC - 1))
                    yt = oy.tile([P, NQ], F32, name="yt", tag="yt")
                    nc.scalar.activation(yt[:], ps[:], AF.Identity, bias=bo_s[:, mc:mc + 1])
                    for t in range(NQT):
                        pt = ot_ps.tile([P, P], F32, name="opt", tag="otp")
                        nc.tensor.transpose(pt[:], yt[:, t * P:(t + 1) * P], ident[:])
                        nc.vector.tensor_add(tgt2[t][:, mc * P:(mc + 1) * P], pt[:],
                                             tgt_r[t][:, mc * P:(mc + 1) * P])
                # this 512-col half of tgt2 is complete: bn_stats now
                for t in range(NQT):
                    nc.vector.bn_stats(out=st2[t][:, mcg, :],
                                       in_=tgt2[t][:, mcg * 512:(mcg + 1) * 512])

        mlp = S.enter_context(tc.tile_pool(name="mlp", bufs=1))
        ln2T = [mlp.tile([P, NQ], F32R, name=f"ln2T{k}") for k in range(KC)]
        with tc.tile_pool(name="ln2w", bufs=4) as ln2w, \
             tc.tile_pool(name="ln2s", bufs=2) as ln2s, \
             tc.tile_pool(name="l2_ps", bufs=4, space="PSUM") as l2_ps:
            ln2 = [ln2s.tile([P, DIM], F32, name=f"ln2_{t}", tag="ln2") for t in range(NQT)]
            for t in range(NQT):
                mv = ln2w.tile([P, nc.vector.BN_AGGR_DIM], F32, name=f"mv2{t}", tag="mv")
                nc.vector.bn_aggr(out=mv[:], in_=st2[t][:])
                rstd = ln2w.tile([P, 1], F32, name=f"rstd2{t}", tag="rstd")
                nc.scalar.activation(out=rstd[:], in_=mv[:, 1:2], func=AF.Sqrt,
                                     bias=eps_t[:], scale=1.0)
                nc.vector.reciprocal(out=rstd[:], in_=rstd[:])
                nc.vector.tensor_scalar(out=ln2[t][:], in0=tgt2[t][:], scalar1=mv[:, 0:1],
                                        scalar2=rstd[:], op0=mybir.AluOpType.subtract,
                                        op1=mybir.AluOpType.mult)
            for t in range(NQT):
                for k in range(KC):
                    pt = l2_ps.tile([P, P], F32, name="l2pt", tag="l2tp")
                    nc.tensor.transpose(pt[:], ln2[t][:, k * P:(k + 1) * P], ident[:])
                    if k % 2 == 0:
                        nc.vector.tensor_copy(ln2T[k][:, t * P:(t + 1) * P], pt[:])
                    else:
                        nc.scalar.activation(ln2T[k][:, t * P:(t + 1) * P], pt[:], AF.Copy)

        # ---------- Phase 4: fc1 (gelu) ----------
        h1T = [mlp.tile([P, NQ], F32R, name=f"h1T{m}") for m in range(HID // P)]
        with tc.tile_pool(name="w1s", bufs=5) as w1s, \
             tc.tile_pool(name="f1_ps", bufs=1, space="PSUM") as f1_ps:
            for grp_i in range(4):
                pss = [f1_ps.tile([P, NQ], F32, name=f"f1p{j}", tag=f"f1_{j}") for j in range(8)]
                for k in range(KC):
                    wt = w1s.tile([P, 1024], F32R, name="w1t", tag="w1")
                    nc.sync.dma_start(out=wt[:], in_=w1[k, grp_i])
                    for j in range(8):
                        nc.tensor.matmul(pss[j][:], wt[:, j * P:(j + 1) * P], ln2T[k][:],
                                         start=(k == 0), stop=(k == KC - 1))
                for j in range(8):
                    hm = grp_i * 8 + j
                    nc.scalar.activation(h1T[hm][:], pss[j][:], AF.Gelu,
                                         bias=b1_s[:, hm:hm + 1])

        # ---------- Phase 5: fc2 + residual + store ----------
        out_sb = [outp.tile([P, DIM], F32, name=f"osb{t}") for t in range(NQT)]
        y2T = [mlp.tile([P, NQ], F32, name=f"y2T{m}") for m in range(KC)]
        with tc.tile_pool(name="w2s", bufs=8) as w2s, \
             tc.tile_pool(name="f2_ps", bufs=1, space="PSUM") as f2_ps, \
             tc.tile_pool(name="y2_ps", bufs=4, space="PSUM") as y2_ps:
            for half in range(2):
                pss = [f2_ps.tile([P, NQ], F32, name=f"f2p{half}_{j}", tag=f"f2_{j}")
                       for j in range(4)]
                for hm in range(HID // P):
                    wt = w2s.tile([P, 512], F32R, name="w2t", tag="w2")
                    nc.sync.dma_start(out=wt[:], in_=w2[half, hm])
                    for j in range(4):
                        nc.tensor.matmul(pss[j][:], wt[:, j * P:(j + 1) * P], h1T[hm][:],
                                         start=(hm == 0), stop=(hm == HID // P - 1))
                for j in range(4):
                    mc = half * 4 + j
                    nc.vector.tensor_scalar_add(y2T[mc][:], pss[j][:], b2_s[:, mc:mc + 1])
                for j in range(4):
                    mc = half * 4 + j
                    for t in range(NQT):
                        pt = y2_ps.tile([P, P], F32, name="y2pt", tag="y2tp")
                        nc.tensor.transpose(pt[:], y2T[mc][:, t * P:(t + 1) * P], ident[:])
                        nc.vector.tensor_add(out_sb[t][:, mc * P:(mc + 1) * P], pt[:],
                                             tgt2[t][:, mc * P:(mc + 1) * P])
            for t in range(NQT):
                nc.sync.dma_start(out=out[t * P:(t + 1) * P, :], in_=out_sb[t][:])

    nc.compile()
    return nc


def _get_nc():
    if "nc" not in _CACHE:
        _CACHE["nc"] = _build()
    return _CACHE["nc"]


def kernel(tgt, emb_motion, ln_g, ln_b, wq, bq, wk, bk, wv, bv, wo, bo, w1, b1, w2, b2):
    from concourse.bass_utils import run_bass_kernel_spmd

    nc = _get_nc()
    f = np.ascontiguousarray
    a32 = lambda x: np.asarray(x, np.float32)

    # fold LN affine (g, b) into wq/w1 and bq/b1 (exact: (xh*g+b)@W = xh@(g*W) + b@W)
    g32, b32 = a32(ln_g), a32(ln_b)
    wq_e = a32(wq) * g32[:, None]
    bq_e = a32(bq) + b32 @ a32(wq)
    w1_e = a32(w1) * g32[:, None]
    b1_e = a32(b1) + b32 @ a32(w1)
    # pretile wk/wv: [1024, 1024] -> [8(kc), 4(quarter), 128, 256]
    wk_t = f(a32(wk).reshape(8, 128, 4, 256).transpose(0, 2, 1, 3))
    wv_t = f(a32(wv).reshape(8, 128, 4, 256).transpose(0, 2, 1, 3))
    w1_t = f(w1_e.reshape(8, 128, 4, 1024).transpose(0, 2, 1, 3))
    w2_t = f(a32(w2).reshape(32, 128, 2, 512).transpose(2, 0, 1, 3))
    bias_pack = np.concatenate([
        bq_e.reshape(8, 128).T, a32(bk).reshape(8, 128).T,
        a32(bo).reshape(8, 128).T, a32(b2).reshape(8, 128).T,
        b1_e.reshape(32, 128).T,
    ], axis=1)
    bias_pack = f(bias_pack.astype(np.float32))

    B = tgt.shape[0]
    in_maps = []
    for c in range(8):
        b, h = divmod(c, 2)
        in_maps.append({
            "tgt": f(a32(tgt[b, h * NQ:(h + 1) * NQ])),
            "emb": f(a32(emb_motion[b]).T),
            "bv": f(a32(bv)),
            "wq": f(wq_e), "wk": wk_t, "wv": wv_t, "wo": f(a32(wo)),
            "w1": w1_t, "w2": w2_t, "bias_pack": bias_pack,
        })
    r = run_bass_kernel_spmd(nc, in_maps, list(range(8)))
    res = np.empty((B, 1024, DIM), np.float32)
    for c in range(8):
        b, h = divmod(c, 2)
        res[b, h * NQ:(h + 1) * NQ] = r.results[c]["out"]
    return res



# revision 10
# speedup vs baseline: 1.3710x; 1.3710x over previous
import sys

sys.path.insert(0, "/opt/trn_rl_repo")
import numpy as np

DIM = 1024
HEADS = 16
HD = 64
HID = 4096
EPS = 1e-5
NQ = 512          # queries per core
NK = 2048
P = 128

SQ = 64.0         # wq scale (fp8 range)
SK = 64.0         # wk scale
SV = 64.0         # wv scale
SO = 64.0         # wo scale
ESC = 0.125 / (SQ * SK)   # exp scale: scores/temp with fp8 scales folded out
OSC = 1.0 / (SV * SO)     # out-proj descale (ctx8 = SV*ctx_normed, wo8 = SO*wo)

_CACHE = {}


def _build():
    import concourse.bacc as bacc
    import concourse.tile as tile
    from concourse import mybir
    from concourse.masks import make_identity
    from contextlib import ExitStack

    F32 = mybir.dt.float32
    BF16 = mybir.dt.bfloat16
    F8 = mybir.dt.float8e4
    AF = mybir.ActivationFunctionType
    DR = mybir.MatmulPerfMode.DoubleRow

    nc = bacc.Bacc(None, target_bir_lowering=False, debug=False)

    tgt = nc.declare_dram_parameter("tgt", [NQ, DIM], F32, isOutput=False)
    # emb8[p, kc, ic, n] = fp8(emb[n, 256*kc + 128*ic + p])
    emb8 = nc.declare_dram_parameter("emb8", [P, 4, 2, NK], F8, isOutput=False)
    # wq8[p, g, io, kc, ic, c(=32h'+j)] = SQ*wq_e[256kc+128ic+p, 256g+64h'+32io+j]
    wq8 = nc.declare_dram_parameter("wq8", [P, 4, 2, 4, 2, P], F8, isOutput=False)
    wk8 = nc.declare_dram_parameter("wk8", [P, 4, 2, 4, 2, P], F8, isOutput=False)
    # wv8[p, g, kc, ic, j(=64hq+d)] = SV*wv[256kc+128ic+p, 256g+j]
    wv8 = nc.declare_dram_parameter("wv8", [P, 4, 4, 2, 256], F8, isOutput=False)
    # wo8[p(0:64), g8, ic, m] = SO*wo[64*(2*g8+ic)+p, m]
    wo8 = nc.declare_dram_parameter("wo8", [64, 8, 2, DIM], F8, isOutput=False)
    # w1b[hg][p][kc, hcol] = bf16(w1_e[128kc+p, 512hg+hcol])
    w1b = nc.declare_dram_parameter("w1b", [8, P, 8, 512], BF16, isOutput=False)
    # w2b[hm][p][m] = bf16(w2[128hm+p, m])
    w2b = nc.declare_dram_parameter("w2b", [HID // P, P, DIM], BF16, isOutput=False)
    # biasf: [128, 48]: cols 0:8 SQ*bq_e by (g,io), 8:16 SK*bk, 16:48 b1_e by hm
    biasf = nc.declare_dram_parameter("biasf", [P, 48], F32, isOutput=False)
    bvb = nc.declare_dram_parameter("bvb", [DIM], BF16, isOutput=False)   # SV*bv
    bob = nc.declare_dram_parameter("bob", [DIM], BF16, isOutput=False)   # bo
    b2b = nc.declare_dram_parameter("b2b", [DIM], BF16, isOutput=False)   # b2
    out = nc.declare_dram_parameter("out", [NQ, DIM], F32, isOutput=True)

    def bcast_ap(vec, n):
        import concourse.bass as bass
        return bass.AP(tensor=vec.tensor, offset=vec.offset, ap=[[0, P], [1, n]])

    with tile.TileContext(nc) as tc, ExitStack() as S:
        const = S.enter_context(tc.tile_pool(name="const", bufs=1))

        identf = const.tile([P, P], F32)
        make_identity(nc, identf)
        identb = const.tile([P, P], BF16)
        nc.scalar.activation(identb[:], identf[:], AF.Copy)
        eps_t = const.tile([P, 1], F32)
        nc.vector.memset(eps_t[:], EPS)

        bp = const.tile([P, 48], F32)
        nc.sync.dma_start(out=bp[:], in_=biasf[:, :])
        bvb_b = const.tile([P, DIM], BF16)
        nc.gpsimd.dma_start(out=bvb_b[:], in_=bcast_ap(bvb[:], DIM))
        bob_b = const.tile([P, DIM], BF16)
        nc.gpsimd.dma_start(out=bob_b[:], in_=bcast_ap(bob[:], DIM))
        b2b_b = const.tile([P, DIM], BF16)
        nc.gpsimd.dma_start(out=b2b_b[:], in_=bcast_ap(b2b[:], DIM))

        # ---- persistent SBUF ----
        perm = S.enter_context(tc.tile_pool(name="perm", bufs=1))
        eT8 = perm.tile([P, 4, 2, NK], F8)          # emb, feature-major fp8
        nc.sync.dma_start(out=eT8[:], in_=emb8[:, :, :, :])
        wk_sb = perm.tile([P, 4, 2, 4, 2, P], F8)
        nc.sync.dma_start(out=wk_sb[:], in_=wk8[:, :, :, :, :, :])
        tgt_t = [perm.tile([P, DIM], F32, name=f"tgt{t}") for t in range(4)]
        for t in range(4):
            nc.sync.dma_start(out=tgt_t[t][:], in_=tgt[t * P:(t + 1) * P, :])
        wv_sb = perm.tile([P, 4, 4, 2, 256], F8)
        nc.sync.dma_start(out=wv_sb[:], in_=wv8[:, :, :, :, :])
        wo_sb = perm.tile([64, 8, 2, DIM], F8)
        nc.sync.dma_start(out=wo_sb[:], in_=wo8[:, :, :, :])

        K8g = [perm.tile([P, 2, NK], F8, name=f"K8_{g}") for g in range(4)]
        Q8g = [perm.tile([P, 2, NQ], F8, name=f"Q8_{g}") for g in range(4)]
        # head 3 of each group sits at partition base 96, which matmul APs
        # can't address -> DMA-shift its 32 partitions down to base 0
        K8h3 = [perm.tile([32, 2, NK], F8, name=f"K8h3_{g}") for g in range(4)]
        Q8h3 = [perm.tile([32, 2, NQ], F8, name=f"Q8h3_{g}") for g in range(4)]
        lnT8 = [perm.tile([P, 2, NQ], F8, name=f"lnT8_{k}") for k in range(4)]
        ctx8 = [perm.tile([64, 2, NQ], F8, name=f"ctx8_{g8}") for g8 in range(8)]
        tgt2 = [perm.tile([P, DIM], F32, name=f"tgt2_{t}") for t in range(4)]
        ln2T = [perm.tile([P, NQ], BF16, name=f"ln2T_{k}") for k in range(8)]
        h1T = [perm.tile([P, NQ], BF16, name=f"h1T_{m}") for m in range(HID // P)]

        def layer_norm_rows(x, y, pool, nm):
            # y = (x - mean)/std rowwise over 1024, y may be bf16
            st = pool.tile([P, 2, nc.vector.BN_STATS_DIM], F32, name=f"st{nm}", tag="st")
            for sg in range(2):
                nc.vector.bn_stats(out=st[:, sg, :], in_=x[:, sg * 512:(sg + 1) * 512])
            mv = pool.tile([P, nc.vector.BN_AGGR_DIM], F32, name=f"mv{nm}", tag="mv")
            nc.vector.bn_aggr(out=mv[:], in_=st[:])
            rstd = pool.tile([P, 1], F32, name=f"rstd{nm}", tag="rstd")
            nc.scalar.activation(out=rstd[:], in_=mv[:, 1:2], func=AF.Sqrt,
                                 bias=eps_t[:], scale=1.0)
            nc.vector.reciprocal(out=rstd[:], in_=rstd[:])
            nc.vector.tensor_scalar(out=y[:], in0=x[:], scalar1=mv[:, 0:1],
                                    scalar2=rstd[:], op0=mybir.AluOpType.subtract,
                                    op1=mybir.AluOpType.mult)

        # ---------- Phase 1: LN1 -> lnT8; kproj -> K8; qproj -> Q8 ----------
        wq_cm = tc.tile_pool(name="wqp", bufs=1)
        wqp = wq_cm.__enter__()
        wq_sb = wqp.tile([P, 4, 2, 4, 2, P], F8)
        nc.sync.dma_start(out=wq_sb[:], in_=wq8[:, :, :, :, :, :])

        ln1_cm = tc.tile_pool(name="ln1", bufs=1)
        ln1 = ln1_cm.__enter__()
        with tc.tile_pool(name="lnw", bufs=4) as lnw, \
             tc.tile_pool(name="kq_ps", bufs=4, space="PSUM") as kq_ps, \
             tc.tile_pool(name="tp_ps", bufs=4, space="PSUM") as tp_ps:
            # kproj: all 4 groups -> K8 (fp8 DoubleRow, contraction 256)
            for g in range(4):
                for io in range(2):
                    for nck in range(4):
                        ps = kq_ps.tile([P, NQ], F32, name="kps", tag="kq")
                        for kc in range(4):
                            nc.tensor.matmul(
                                ps[:], wk_sb[:, g, io, kc, :, :],
                                eT8[:, kc, :, nck * NQ:(nck + 1) * NQ],
                                start=(kc == 0), stop=(kc == 3), perf_mode=DR)
                        cv_eng = nc.scalar if g < 2 else nc.vector
                        if g < 2:
                            nc.scalar.activation(
                                K8g[g][:, io, nck * NQ:(nck + 1) * NQ], ps[:],
                                AF.Identity, bias=bp[:, 8 + 2 * g + io:9 + 2 * g + io])
                        else:
                            nc.vector.tensor_scalar_add(
                                K8g[g][:, io, nck * NQ:(nck + 1) * NQ], ps[:],
                                bp[:, 8 + 2 * g + io:9 + 2 * g + io])
                nc.sync.dma_start(out=K8h3[g][:], in_=K8g[g][96:128, :, :])

            # LN1 rows (bf16) while kproj runs on PE
            ln1r = [ln1.tile([P, DIM], BF16, name=f"ln1r{t}") for t in range(4)]
            for t in range(4):
                layer_norm_rows(tgt_t[t], ln1r[t], lnw, f"a{t}")
            # transpose LN1 rows -> lnT8 (bf16 transpose, fp8 out on copy)
            for t in range(4):
                for kc in range(4):
                    for ic in range(2):
                        pt = tp_ps.tile([P, P], BF16, name="pt", tag="tp")
                        f0 = 256 * kc + 128 * ic
                        nc.tensor.transpose(pt[:], ln1r[t][:, f0:f0 + P], identb[:])
                        nc.vector.tensor_copy(lnT8[kc][:, ic, t * P:(t + 1) * P], pt[:])
            # qproj -> Q8
            for g in range(4):
                for io in range(2):
                    ps = kq_ps.tile([P, NQ], F32, name="qps", tag="kq")
                    for kc in range(4):
                        nc.tensor.matmul(ps[:], wq_sb[:, g, io, kc, :, :], lnT8[kc][:],
                                         start=(kc == 0), stop=(kc == 3), perf_mode=DR)
                    nc.scalar.activation(Q8g[g][:, io, :], ps[:], AF.Identity,
                                         bias=bp[:, 2 * g + io:2 * g + io + 1])
                nc.sync.dma_start(out=Q8h3[g][:], in_=Q8g[g][96:128, :, :])
        ln1_cm.__exit__(None, None, None)
        wq_cm.__exit__(None, None, None)

        # ---------- Phase 2: attention (fp8 DoubleRow) ----------
        with tc.tile_pool(name="v8p", bufs=2) as v8p, \
             tc.tile_pool(name="exp8", bufs=3) as exp8, \
             tc.tile_pool(name="sm", bufs=2) as sm, \
             tc.tile_pool(name="vps", bufs=2, space="PSUM") as vps, \
             tc.tile_pool(name="scp", bufs=2, space="PSUM") as scp, \
             tc.tile_pool(name="cxp", bufs=2, space="PSUM") as cxp:

            v8 = {}

            def emit_vproj(g):
                # v8[g][t]: [128, 2(ic=kvt parity), 4(hq), 65] fp8; col 64 = ones
                v8[g] = [v8p.tile([P, 2, 4, 68], F8, name=f"v8_{g}_{t}", tag=f"v{t}")
                         for t in range(8)]
                for t in range(8):
                    ps = vps.tile([P, NQ], F32, name="vp", tag="vp")
                    for half in range(2):   # kvt = 2t + half
                        kvt = 2 * t + half
                        for kc in range(4):
                            nc.tensor.matmul(
                                ps[:, half * 256:(half + 1) * 256],
                                eT8[:, kc, :, kvt * P:(kvt + 1) * P],
                                wv_sb[:, g, kc, :, :],
                                start=(kc == 0), stop=(kc == 3), perf_mode=DR)
                    for half in range(2):
                        nc.vector.tensor_tensor(
                            out=v8[g][t][:, half, :, 0:64],
                            in0=ps[:, half * 256:(half + 1) * 256].rearrange(
                                "p (h d) -> p h d", d=64),
                            in1=bvb_b[:, 256 * g:256 * (g + 1)].rearrange(
                                "p (h d) -> p h d", d=64),
                            op=mybir.AluOpType.add)
                    nc.gpsimd.memset(v8[g][t][:, :, :, 64:65], 1.0)

            emit_vproj(0)
            for g in range(4):
                for h in range(4):
                    head = 4 * g + h
                    g8, ih = divmod(head, 2)
                    cps = cxp.tile([P, NQ], F32, name="cps", tag="cps")
                    if h < 3:
                        Ksrc, Qsrc, pb = K8g[g], Q8g[g], 32 * h
                    else:
                        Ksrc, Qsrc, pb = K8h3[g], Q8h3[g], 0
                    for t in range(8):
                        sc = scp.tile([P, 2 * NQ], F32, name="sc", tag="sc")
                        for half in range(2):
                            kvt = 2 * t + half
                            nc.tensor.matmul(
                                sc[:, half * NQ:(half + 1) * NQ],
                                Ksrc[pb:pb + 32, :, kvt * P:(kvt + 1) * P],
                                Qsrc[pb:pb + 32, :, :],
                                start=True, stop=True, perf_mode=DR)
                        ex = exp8.tile([P, 2 * NQ], F8, name="ex", tag="ex")
                        nc.scalar.activation(ex[:], sc[:], AF.Exp, scale=ESC)
                        nc.tensor.matmul(
                            cps[0:65, :], v8[g][t][:, :, h, 0:65],
                            ex[:].rearrange("p (i n) -> p i n", i=2),
                            start=(t == 0), stop=(t == 7), perf_mode=DR)
                    # interleave next quarter's vproj behind head 0's PE work
                    if h == 0 and g < 3:
                        emit_vproj(g + 1)
                    # softmax tail: normalize by denominator (row 64)
                    rl = sm.tile([P, NQ], F32, name="rl", tag="rl")
                    nc.vector.reciprocal(out=rl[64:65, :], in_=cps[64:65, :])
                    rl0 = sm.tile([1, NQ], F32, name="rl0", tag="rl0")
                    nc.sync.dma_start(out=rl0[0:1, :], in_=rl[64:65, :])
                    bcs = sm.tile([64, NQ], F32, name="bcs", tag="bcs")
                    nc.gpsimd.partition_broadcast(bcs[:], rl0[0:1, :], channels=64)
                    nc.vector.tensor_tensor(
                        out=ctx8[g8][0:64, ih, :], in0=cps[0:64, :], in1=bcs[:],
                        op=mybir.AluOpType.mult)

        # ---------- Phase 3: out-proj + residual -> tgt2; LN2 -> ln2T ----------
        ln2_cm = tc.tile_pool(name="ln2", bufs=1)
        ln2 = ln2_cm.__enter__()
        ln2r = [ln2.tile([P, DIM], BF16, name=f"ln2r{t}") for t in range(4)]
        with tc.tile_pool(name="ln2w", bufs=4) as ln2w, \
             tc.tile_pool(name="o_ps", bufs=2, space="PSUM") as o_ps, \
             tc.tile_pool(name="l2_ps", bufs=4, space="PSUM") as l2_ps:
            for t in range(4):
                # fold +bo into residual before the STT add
                nc.vector.tensor_tensor(out=tgt_t[t][:], in0=tgt_t[t][:],
                                        in1=bob_b[:], op=mybir.AluOpType.add)
                ps = o_ps.tile([P, DIM], F32, name="ops", tag="op")
                for g8 in range(8):
                    for mh in range(2):
                        nc.tensor.matmul(ps[:, mh * 512:(mh + 1) * 512],
                                         ctx8[g8][:, :, t * P:(t + 1) * P],
                                         wo_sb[:, g8, :, mh * 512:(mh + 1) * 512],
                                         start=(g8 == 0), stop=(g8 == 7), perf_mode=DR)
                nc.vector.scalar_tensor_tensor(
                    out=tgt2[t][:], in0=ps[:], scalar=OSC, in1=tgt_t[t][:],
                    op0=mybir.AluOpType.mult, op1=mybir.AluOpType.add)
                layer_norm_rows(tgt2[t], ln2r[t], ln2w, f"b{t}")
                # b2 folded into tgt2 AFTER stats are taken (fc2 residual)
                nc.vector.tensor_tensor(out=tgt2[t][:], in0=tgt2[t][:],
                                        in1=b2b_b[:], op=mybir.AluOpType.add)
                for k in range(8):
                    pt = l2_ps.tile([P, P], BF16, name="l2pt", tag="l2tp")
                    nc.tensor.transpose(pt[:], ln2r[t][:, k * P:(k + 1) * P], identb[:])
                    nc.vector.tensor_copy(ln2T[k][:, t * P:(t + 1) * P], pt[:])
        ln2_cm.__exit__(None, None, None)

        # ---------- Phase 4: fc1 (bf16) + gelu ----------
        with tc.tile_pool(name="w1s", bufs=2) as w1s, \
             tc.tile_pool(name="f1_ps", bufs=2, space="PSUM") as f1_ps:
            for hg in range(8):
                wt = w1s.tile([P, 8, 512], BF16, name="w1t", tag="w1")
                nc.sync.dma_start(out=wt[:], in_=w1b[hg])
                pss = [f1_ps.tile([P, NQ], F32, name=f"f1p{hc}", tag=f"f1_{hc}")
                       for hc in range(4)]
                for kc in range(8):
                    for hc in range(4):
                        nc.tensor.matmul(pss[hc][:], wt[:, kc, hc * P:(hc + 1) * P],
                                         ln2T[kc][:], start=(kc == 0), stop=(kc == 7))
                for hc in range(4):
                    hm = hg * 4 + hc
                    nc.scalar.activation(h1T[hm][:], pss[hc][:], AF.Gelu,
                                         bias=bp[:, 16 + hm:17 + hm])

        # ---------- Phase 5: fc2 (bf16) + residual + store ----------
        with tc.tile_pool(name="w2s", bufs=4) as w2s, \
             tc.tile_pool(name="f2_ps", bufs=1, space="PSUM") as f2_ps:
            pss = [f2_ps.tile([P, DIM], F32, name=f"f2p{t}", tag=f"f2_{t}")
                   for t in range(4)]
            for hm in range(HID // P):
                wt = w2s.tile([P, DIM], BF16, name="w2t", tag="w2")
                nc.sync.dma_start(out=wt[:], in_=w2b[hm])
                for t in range(4):
                    for mh in range(2):
                        nc.tensor.matmul(pss[t][:, mh * 512:(mh + 1) * 512],
                                         h1T[hm][:, t * P:(t + 1) * P],
                                         wt[:, mh * 512:(mh + 1) * 512],
                                         start=(hm == 0), stop=(hm == HID // P - 1))
            for t in range(4):
                nc.vector.tensor_tensor(out=tgt_t[t][:], in0=pss[t][:],
                                        in1=tgt2[t][:], op=mybir.AluOpType.add)
                nc.sync.dma_start(out=out[t * P:(t + 1) * P, :], in_=tgt_t[t][:])

    nc.compile()
    return nc


def _get_nc():
    if "nc" not in _CACHE:
        _CACHE["nc"] = _build()
    return _CACHE["nc"]


def kernel(tgt, emb_motion, ln_g, ln_b, wq, bq, wk, bk, wv, bv, wo, bo, w1, b1, w2, b2):
    import ml_dtypes
    from concourse.bass_utils import run_bass_kernel_spmd

    nc = _get_nc()
    f = np.ascontiguousarray
    a32 = lambda x: np.asarray(x, np.float32)
    FP8 = ml_dtypes.float8_e4m3
    BF = ml_dtypes.bfloat16

    def q8(x):
        return np.clip(x, -440.0, 440.0).astype(FP8)

    # fold LN affine into wq/w1 (exact: (xh*g+b)@W = xh@(g*W) + (b@W))
    g32, b32 = a32(ln_g), a32(ln_b)
    wq_e = a32(wq) * g32[:, None]
    bq_e = a32(bq) + b32 @ a32(wq)
    w1_e = a32(w1) * g32[:, None]
    b1_e = a32(b1) + b32 @ a32(w1)

    # wq8/wk8: [p, g, io, kc, ic, c] = S*W[256kc+128ic+p, 256g+64h'+32io+j], c=32h'+j
    def pack_qk(W, S):
        A = (a32(W) * S).reshape(4, 2, 128, 4, 4, 2, 32)  # [kc, ic, p, g, h', io, j]
        return q8(f(A.transpose(2, 3, 5, 0, 1, 4, 6).reshape(128, 4, 2, 4, 2, 128)))

    wq8 = pack_qk(wq_e, SQ)
    wk8 = pack_qk(wk, SK)
    # wv8: [p, g, kc, ic, j] = SV*wv[256kc+128ic+p, 256g+j]
    A = (a32(wv) * SV).reshape(4, 2, 128, 4, 256)          # [kc, ic, p, g, j]
    wv8 = q8(f(A.transpose(2, 3, 0, 1, 4)))
    # wo8: [p, g8, ic, m] = SO*wo[64*(2g8+ic)+p, m]
    A = (a32(wo) * SO).reshape(8, 2, 64, 1024)             # [g8, ic, p, m]
    wo8 = q8(f(A.transpose(2, 0, 1, 3)))
    # w1b: [hg, p, kc, hcol] ; w2b: [hm, p, m]
    A = w1_e.reshape(8, 128, 8, 512)                       # [kc, p, hg, hcol]
    w1bh = f(A.transpose(2, 1, 0, 3)).astype(BF)
    w2bh = f(a32(w2).reshape(32, 128, 1024)).astype(BF)

    # biasf [128, 48]
    biasf = np.zeros((128, 48), np.float32)
    bq_s = (SQ * bq_e).reshape(4, 4, 2, 32)                # [g, h', io, j]
    bk_s = (SK * a32(bk)).reshape(4, 4, 2, 32)
    for g in range(4):
        for io in range(2):
            biasf[:, 2 * g + io] = bq_s[g, :, io, :].reshape(128)
            biasf[:, 8 + 2 * g + io] = bk_s[g, :, io, :].reshape(128)
    biasf[:, 16:48] = b1_e.reshape(32, 128).T

    bvb = (SV * a32(bv)).astype(BF)
    bob = a32(bo).astype(BF)
    b2b = a32(b2).astype(BF)

    B = tgt.shape[0]
    emb8_by_b = {}
    for b in range(B):
        # emb8[p, kc, ic, n] = fp8(emb[n, 256kc+128ic+p])
        E = a32(emb_motion[b]).T.reshape(4, 2, 128, NK)    # [kc, ic, p, n]
        emb8_by_b[b] = q8(f(E.transpose(2, 0, 1, 3)))

    in_maps = []
    for c in range(8):
        b, h = divmod(c, 2)
        in_maps.append({
            "tgt": f(a32(tgt[b, h * NQ:(h + 1) * NQ])),
            "emb8": emb8_by_b[b],
            "wq8": wq8, "wk8": wk8, "wv8": wv8, "wo8": wo8,
            "w1b": w1bh, "w2b": w2bh,
            "biasf": biasf, "bvb": bvb, "bob": bob, "b2b": b2b,
        })
    r = run_bass_kernel_spmd(nc, in_maps, list(range(8)))
    res = np.empty((B, 1024, DIM), np.float32)
    for c in range(8):
        b, h = divmod(c, 2)
        res[b, h * NQ:(h + 1) * NQ] = r.results[c]["out"]
    return res


# revision 19
# speedup vs baseline: 1.4610x; 1.0656x over previous
import sys

sys.path.insert(0, "/opt/trn_rl_repo")
import numpy as np

DIM = 1024
HEADS = 16
HD = 64
HID = 4096
EPS = 1e-5
NQ = 512          # queries per core
NK = 2048
P = 128

SQ = 64.0         # wq scale (fp8 range)
SK = 64.0         # wk scale
SV = 64.0         # wv scale
SO = 64.0         # wo scale
ESC = 0.125 / (SQ * SK)   # exp scale: scores/temp with fp8 scales folded out
SCH_A = 12102203.161561486 * ESC   # 2^23/ln2, folded with ESC
SCH_B = 1064986823.0               # 127*2^23 - 366393 (min-RMS bias)
OSC = 1.0 / (SV * SO)     # out-proj descale (ctx8 = SV*ctx_normed, wo8 = SO*wo)

_CACHE = {}


def _build():
    import concourse.bacc as bacc
    import concourse.tile as tile
    from concourse import mybir
    from concourse.masks import make_identity
    from contextlib import ExitStack

    F32 = mybir.dt.float32
    BF16 = mybir.dt.bfloat16
    F8 = mybir.dt.float8e4
    AF = mybir.ActivationFunctionType
    DR = mybir.MatmulPerfMode.DoubleRow

    nc = bacc.Bacc(None, target_bir_lowering=False, debug=False)

    tgt = nc.declare_dram_parameter("tgt", [NQ, DIM], F32, isOutput=False)
    # emb8[p, kc, ic, n] = fp8(emb[n, 256*kc + 128*ic + p])
    emb8 = nc.declare_dram_parameter("emb8", [P, 4, 2, NK], F8, isOutput=False)
    # wq8[p, g, io, kc, ic, c(=32h'+j)] = SQ*wq_e[256kc+128ic+p, 256g+64h'+32io+j]
    wq8 = nc.declare_dram_parameter("wq8", [P, 4, 2, 4, 2, P], F8, isOutput=False)
    wk8 = nc.declare_dram_parameter("wk8", [P, 4, 2, 4, 2, P], F8, isOutput=False)
    # wv8[p, g, kc, ic, j(=64hq+d)] = SV*wv[256kc+128ic+p, 256g+j]
    wv8 = nc.declare_dram_parameter("wv8", [P, 4, 4, 2, 256], F8, isOutput=False)
    # wo8[p(0:64), g8, ic, m] = SO*wo[64*(2*g8+ic)+p, m]
    wo8 = nc.declare_dram_parameter("wo8", [64, 8, 2, DIM], F8, isOutput=False)
    # w1b[hg][p][kc, hcol] = bf16(w1_e[128kc+p, 512hg+hcol])
    w1b = nc.declare_dram_parameter("w1b", [8, P, 8, 512], BF16, isOutput=False)
    # w2b[hm][p][m] = bf16(w2[128hm+p, m])
    w2b = nc.declare_dram_parameter("w2b", [HID // P, P, DIM], BF16, isOutput=False)
    # biasf: [128, 48]: cols 0:8 SQ*bq_e by (g,io), 8:16 SK*bk, 16:48 b1_e by hm
    biasf = nc.declare_dram_parameter("biasf", [P, 48], F32, isOutput=False)
    bvb = nc.declare_dram_parameter("bvb", [2 * DIM], F8, isOutput=False)  # SV*bv, dup x2
    bob = nc.declare_dram_parameter("bob", [DIM], BF16, isOutput=False)   # bo
    b2b = nc.declare_dram_parameter("b2b", [DIM], BF16, isOutput=False)   # b2
    out = nc.declare_dram_parameter("out", [NQ, DIM], F32, isOutput=True)

    def bcast_ap(vec, n):
        import concourse.bass as bass
        return bass.AP(tensor=vec.tensor, offset=vec.offset, ap=[[0, P], [1, n]])

    with tile.TileContext(nc) as tc, ExitStack() as S:
        const = S.enter_context(tc.tile_pool(name="const", bufs=1))

        identf = const.tile([P, P], F32)
        make_identity(nc, identf)
        identb = const.tile([P, P], BF16)
        nc.scalar.activation(identb[:], identf[:], AF.Copy)
        eps_t = const.tile([P, 1], F32)
        nc.vector.memset(eps_t[:], EPS)

        bp = const.tile([P, 48], F32)
        bvb_b = const.tile([P, 2 * DIM], F8)
        bob_b = const.tile([P, DIM], BF16)
        b2b_b = const.tile([P, DIM], BF16)

        # ---- persistent SBUF ----
        perm = S.enter_context(tc.tile_pool(name="perm", bufs=1))
        eT8 = perm.tile([P, 4, 2, NK], F8)          # emb, feature-major fp8
        wk_sb = perm.tile([P, 4, 2, 4, 2, P], F8)
        tgt_t = [perm.tile([P, DIM], F32, name=f"tgt{t}") for t in range(4)]
        nc.sync.dma_start(out=wk_sb[:, 0], in_=wk8[:, 0])
        for kc in range(4):
            nc.sync.dma_start(out=eT8[:, kc, :, :], in_=emb8[:, kc, :, :])
        for g in range(1, 4):
            nc.sync.dma_start(out=wk_sb[:, g], in_=wk8[:, g])
        for t in range(4):
            nc.sync.dma_start(out=tgt_t[t][:], in_=tgt[t * P:(t + 1) * P, :])
        nc.sync.dma_start(out=bp[:], in_=biasf[:, :])
        wv_sb = perm.tile([P, 4, 4, 2, 256], F8)
        nc.sync.dma_start(out=wv_sb[:], in_=wv8[:, :, :, :, :])
        wo_sb = perm.tile([64, 8, 2, DIM], F8)
        nc.sync.dma_start(out=wo_sb[:], in_=wo8[:, :, :, :])
        nc.sync.dma_start(out=bvb_b[:], in_=bcast_ap(bvb[:], 2 * DIM))
        nc.sync.dma_start(out=bob_b[:], in_=bcast_ap(bob[:], DIM))
        nc.sync.dma_start(out=b2b_b[:], in_=bcast_ap(b2b[:], DIM))

        K8g = [perm.tile([P, 2, NK], F8, name=f"K8_{g}") for g in range(4)]
        Q8g = [perm.tile([P, 2, NQ], F8, name=f"Q8_{g}") for g in range(4)]
        # head 3 of each group sits at partition base 96, which matmul APs
        # can't address -> DMA-shift its 32 partitions down to base 0
        K8h3 = [perm.tile([32, 2, NK], F8, name=f"K8h3_{g}") for g in range(4)]
        Q8h3 = [perm.tile([32, 2, NQ], F8, name=f"Q8h3_{g}") for g in range(4)]
        lnT8 = [perm.tile([P, 2, NQ], F8, name=f"lnT8_{k}") for k in range(4)]
        ctx8 = [perm.tile([64, 2, NQ], F8, name=f"ctx8_{g8}") for g8 in range(8)]
        tgt2 = [perm.tile([P, DIM], F32, name=f"tgt2_{t}") for t in range(4)]
        ln2T = [[perm.tile([P, P], BF16, name=f"ln2T_{k}_{t}") for t in range(4)]
                for k in range(8)]
        h1T = [perm.tile([P, NQ], BF16, name=f"h1T_{m}") for m in range(HID // P)]

        def layer_norm_rows(x, y, pool, nm):
            # y = (x - mean)/std rowwise over 1024, y may be bf16
            st = pool.tile([P, 2, nc.vector.BN_STATS_DIM], F32, name=f"st{nm}", tag="st")
            for sg in range(2):
                nc.vector.bn_stats(out=st[:, sg, :], in_=x[:, sg * 512:(sg + 1) * 512])
            mv = pool.tile([P, nc.vector.BN_AGGR_DIM], F32, name=f"mv{nm}", tag="mv")
            nc.vector.bn_aggr(out=mv[:], in_=st[:])
            rstd = pool.tile([P, 1], F32, name=f"rstd{nm}", tag="rstd")
            nc.scalar.activation(out=rstd[:], in_=mv[:, 1:2], func=AF.Sqrt,
                                 bias=eps_t[:], scale=1.0)
            nc.vector.reciprocal(out=rstd[:], in_=rstd[:])
            nb = pool.tile([P, 1], F32, name=f"nb{nm}", tag="nb")
            nc.vector.tensor_scalar(out=nb[:], in0=mv[:, 0:1], scalar1=rstd[:],
                                    scalar2=-1.0, op0=mybir.AluOpType.mult,
                                    op1=mybir.AluOpType.mult)
            nc.scalar.activation(out=y[:], in_=x[:], func=AF.Identity,
                                 bias=nb[:], scale=rstd[:])

        # ---------- Phase 1 + 2: projections and attention, software-pipelined ----------
        kq_cm = tc.tile_pool(name="kq_ps", bufs=2, space="PSUM")
        kq_ps = kq_cm.__enter__()

        ln1_cm = tc.tile_pool(name="ln1", bufs=1)
        ln1 = ln1_cm.__enter__()

        wq_cm = tc.tile_pool(name="wqp", bufs=1)
        wqp = wq_cm.__enter__()
        wq_sb = wqp.tile([P, 4, 2, 4, 2, P], F8)
        nc.sync.dma_start(out=wq_sb[:], in_=wq8[:, :, :, :, :, :])

        def emit_kproj(g, ios=(0, 1)):
            # kproj group g -> K8g[g] (fp8 DoubleRow, contraction 256)
            # group 0 converts on Act (pre-phase); later groups on DVE
            for io in ios:
                for nck in range(4):
                    ps = kq_ps.tile([P, NQ], F32, name="kps", tag="kq")
                    for kc in range(4):
                        nc.tensor.matmul(
                            ps[:], wk_sb[:, g, io, kc, :, :],
                            eT8[:, kc, :, nck * NQ:(nck + 1) * NQ],
                            start=(kc == 0), stop=(kc == 3), perf_mode=DR)
                    if g == 0:
                        nc.scalar.activation(
                            K8g[g][:, io, nck * NQ:(nck + 1) * NQ], ps[:],
                            AF.Identity, bias=bp[:, 8 + 2 * g + io:9 + 2 * g + io])
                    else:
                        nc.vector.tensor_scalar_add(
                            K8g[g][:, io, nck * NQ:(nck + 1) * NQ], ps[:],
                            bp[:, 8 + 2 * g + io:9 + 2 * g + io])
            if ios[-1] == 1:
                nc.sync.dma_start(out=K8h3[g][:], in_=K8g[g][96:128, :, :])

        emit_kproj(0)
        with tc.tile_pool(name="lnw", bufs=4) as lnw, \
             tc.tile_pool(name="tp_ps", bufs=4, space="PSUM") as tp_ps:
            # LN1 rows (bf16) on DVE while kproj g0 runs on PE
            ln1r = [ln1.tile([P, DIM], BF16, name=f"ln1r{t}") for t in range(4)]
            for t in range(4):
                layer_norm_rows(tgt_t[t], ln1r[t], lnw, f"a{t}")
            for t in range(4):
                for kc in range(4):
                    for ic in range(2):
                        pt = tp_ps.tile([P, P], BF16, name="pt", tag="tp")
                        f0 = 256 * kc + 128 * ic
                        nc.tensor.transpose(pt[:], ln1r[t][:, f0:f0 + P], identb[:])
                        nc.vector.tensor_copy(lnT8[kc][:, ic, t * P:(t + 1) * P], pt[:])
            # qproj -> Q8 (converts on Act)
            for g in range(4):
                for io in range(2):
                    ps = kq_ps.tile([P, NQ], F32, name="qps", tag="kq")
                    for kc in range(4):
                        nc.tensor.matmul(ps[:], wq_sb[:, g, io, kc, :, :], lnT8[kc][:],
                                         start=(kc == 0), stop=(kc == 3), perf_mode=DR)
                    nc.scalar.activation(Q8g[g][:, io, :], ps[:], AF.Identity,
                                         bias=bp[:, 2 * g + io:2 * g + io + 1])
                nc.sync.dma_start(out=Q8h3[g][:], in_=Q8g[g][96:128, :, :])
        wq_cm.__exit__(None, None, None)
        ln1_cm.__exit__(None, None, None)

        # ---------- attention (fp8 DoubleRow), kproj/vproj g>=1 interleaved ----------
        with tc.tile_pool(name="v8p", bufs=2) as v8p, \
             tc.tile_pool(name="exp8", bufs=3) as exp8, \
             tc.tile_pool(name="sm", bufs=2) as sm, \
             tc.tile_pool(name="scp", bufs=2, space="PSUM") as scp, \
             tc.tile_pool(name="cxp", bufs=2, space="PSUM") as cxp:

            v8 = {}

            def emit_vproj(g, ts):
                # v8[g][t]: [128, 2(ic=kvt parity), 4(hq), 68] fp8; col 64 = ones
                if g not in v8:
                    v8[g] = [v8p.tile([P, 2, 4, 68], F8, name=f"v8_{g}_{t}", tag=f"v{t}")
                             for t in range(8)]
                for t in ts:
                    ps = kq_ps.tile([P, NQ], F32, name="vp", tag="kq")
                    for half in range(2):   # kvt = 2t + half
                        kvt = 2 * t + half
                        for kc in range(4):
                            nc.tensor.matmul(
                                ps[:, half * 256:(half + 1) * 256],
                                eT8[:, kc, :, kvt * P:(kvt + 1) * P],
                                wv_sb[:, g, kc, :, :],
                                start=(kc == 0), stop=(kc == 3), perf_mode=DR)
                    nc.vector.tensor_tensor(
                        out=v8[g][t][:, :, :, 0:64],
                        in0=ps[:].rearrange("p (i h d) -> p i h d", i=2, h=4),
                        in1=bvb_b[:, 512 * g:512 * (g + 1)].rearrange(
                            "p (i h d) -> p i h d", i=2, h=4),
                        op=mybir.AluOpType.add)
                    nc.gpsimd.memset(v8[g][t][:, :, :, 64:65], 1.0)

            emit_vproj(0, range(8))
            for g in range(4):
                for h in range(4):
                    head = 4 * g + h
                    g8, ih = divmod(head, 2)
                    cps = cxp.tile([P, NQ], F32, name="cps", tag="cps")
                    if h < 3:
                        Ksrc, Qsrc, pb = K8g[g], Q8g[g], 32 * h
                    else:
                        Ksrc, Qsrc, pb = K8h3[g], Q8h3[g], 0
                    for t in range(8):
                        sc = scp.tile([P, 2 * NQ], F32, name="sc", tag="sc")
                        for half in range(2):
                            kvt = 2 * t + half
                            nc.tensor.matmul(
                                sc[:, half * NQ:(half + 1) * NQ],
                                Ksrc[pb:pb + 32, :, kvt * P:(kvt + 1) * P],
                                Qsrc[pb:pb + 32, :, :],
                                start=True, stop=True, perf_mode=DR)
                        ex = exp8.tile([P, 2 * NQ], F8, name="ex", tag="ex")
                        nc.scalar.activation(ex[:], sc[:], AF.Exp, scale=ESC)
                        nc.tensor.matmul(
                            cps[0:65, :], v8[g][t][:, :, h, 0:65],
                            ex[:].rearrange("p (i n) -> p i n", i=2),
                            start=(t == 0), stop=(t == 7), perf_mode=DR)
                    # interleave next quarter's vproj/kproj behind this quarter
                    if g < 3:
                        if h == 0:
                            emit_vproj(g + 1, range(0, 4))
                        elif h == 1:
                            emit_vproj(g + 1, range(4, 8))
                        elif h == 2:
                            emit_kproj(g + 1, (0,))
                        else:
                            emit_kproj(g + 1, (1,))
                    # softmax tail: normalize by denominator (row 64)
                    rl = sm.tile([P, NQ], F32, name="rl", tag="rl")
                    nc.vector.reciprocal(out=rl[64:65, :], in_=cps[64:65, :])
                    rl0 = sm.tile([1, NQ], F32, name="rl0", tag="rl0")
                    nc.sync.dma_start(out=rl0[0:1, :], in_=rl[64:65, :])
                    bcs = sm.tile([64, NQ], F32, name="bcs", tag="bcs")
                    nc.gpsimd.partition_broadcast(bcs[:], rl0[0:1, :], channels=64)
                    nc.vector.tensor_tensor(
                        out=ctx8[g8][0:64, ih, :], in0=cps[0:64, :], in1=bcs[:],
                        op=mybir.AluOpType.mult)

        kq_cm.__exit__(None, None, None)

        # ---------- Phase 3: out-proj + residual -> tgt2; LN2 -> ln2T ----------
        ln2_cm = tc.tile_pool(name="ln2", bufs=1)
        ln2 = ln2_cm.__enter__()
        ln2r = [ln2.tile([P, DIM], BF16, name=f"ln2r{t}") for t in range(4)]
        with tc.tile_pool(name="ln2w", bufs=4) as ln2w, \
             tc.tile_pool(name="o_ps", bufs=1, space="PSUM") as o_ps, \
             tc.tile_pool(name="l2_ps", bufs=4, space="PSUM") as l2_ps:
            for t in range(4):
                # fold +bo into residual before the STT add
                nc.gpsimd.tensor_tensor(out=tgt_t[t][:], in0=tgt_t[t][:],
                                        in1=bob_b[:], op=mybir.AluOpType.add)
                ps = o_ps.tile([P, DIM], F32, name="ops", tag="op")
                for g8 in range(8):
                    for mh in range(2):
                        nc.tensor.matmul(ps[:, mh * 512:(mh + 1) * 512],
                                         ctx8[g8][:, :, t * P:(t + 1) * P],
                                         wo_sb[:, g8, :, mh * 512:(mh + 1) * 512],
                                         start=(g8 == 0), stop=(g8 == 7), perf_mode=DR)
                nc.vector.scalar_tensor_tensor(
                    out=tgt2[t][:], in0=ps[:], scalar=OSC, in1=tgt_t[t][:],
                    op0=mybir.AluOpType.mult, op1=mybir.AluOpType.add)
                layer_norm_rows(tgt2[t], ln2r[t], ln2w, f"b{t}")
                # b2 folded into tgt2 AFTER stats are taken (fc2 residual)
                nc.gpsimd.tensor_tensor(out=tgt2[t][:], in0=tgt2[t][:],
                                        in1=b2b_b[:], op=mybir.AluOpType.add)
                for k in range(8):
                    pt = l2_ps.tile([P, P], BF16, name="l2pt", tag="l2tp")
                    nc.tensor.transpose(pt[:], ln2r[t][:, k * P:(k + 1) * P], identb[:])
                    nc.scalar.activation(ln2T[k][t][:], pt[:], AF.Copy)
        ln2_cm.__exit__(None, None, None)

        # ---------- Phase 4: fc1 (bf16) + gelu ----------
        with tc.tile_pool(name="w1s", bufs=2) as w1s, \
             tc.tile_pool(name="f1_ps", bufs=2, space="PSUM") as f1_ps:
            for hg in range(8):
                wt = w1s.tile([P, 8, 512], BF16, name="w1t", tag="w1")
                nc.sync.dma_start(out=wt[:], in_=w1b[hg])
                for sub in range(2):
                    pss = [f1_ps.tile([P, NQ], F32, name=f"f1p{j}", tag=f"f1_{j}")
                           for j in range(2)]
                    for qt in range(4):
                        for kc in range(8):
                            for j in range(2):
                                hc = 2 * sub + j
                                nc.tensor.matmul(
                                    pss[j][:, qt * P:(qt + 1) * P],
                                    wt[:, kc, hc * P:(hc + 1) * P],
                                    ln2T[kc][qt][:], start=(kc == 0), stop=(kc == 7))
                    for j in range(2):
                        hm = hg * 4 + 2 * sub + j
                        nc.scalar.activation(h1T[hm][:], pss[j][:], AF.Gelu,
                                             bias=bp[:, 16 + hm:17 + hm])

        # ---------- Phase 5: fc2 (bf16) + residual + store ----------
        with tc.tile_pool(name="w2s", bufs=4) as w2s, \
             tc.tile_pool(name="f2_ps", bufs=1, space="PSUM") as f2_ps:
            pss = [f2_ps.tile([P, DIM], F32, name=f"f2p{t}", tag=f"f2_{t}")
                   for t in range(4)]
            for hm in range(HID // P):
                wt = w2s.tile([P, DIM], BF16, name="w2t", tag="w2")
                nc.sync.dma_start(out=wt[:], in_=w2b[hm])
                for t in range(4):
                    for mh in range(2):
                        nc.tensor.matmul(pss[t][:, mh * 512:(mh + 1) * 512],
                                         h1T[hm][:, t * P:(t + 1) * P],
                                         wt[:, mh * 512:(mh + 1) * 512],
                                         start=(hm == 0), stop=(hm == HID // P - 1))
            for t in range(4):
                nc.vector.tensor_tensor(out=tgt_t[t][:], in0=pss[t][:],
                                        in1=tgt2[t][:], op=mybir.AluOpType.add)
                nc.sync.dma_start(out=out[t * P:(t + 1) * P, :], in_=tgt_t[t][:])

    nc.compile()
    return nc


def _get_nc():
    if "nc" not in _CACHE:
        _CACHE["nc"] = _build()
    return _CACHE["nc"]


def kernel(tgt, emb_motion, ln_g, ln_b, wq, bq, wk, bk, wv, bv, wo, bo, w1, b1, w2, b2):
    import ml_dtypes
    from concourse.bass_utils import run_bass_kernel_spmd

    nc = _get_nc()
    f = np.ascontiguousarray
    a32 = lambda x: np.asarray(x, np.float32)
    FP8 = ml_dtypes.float8_e4m3
    BF = ml_dtypes.bfloat16

    def q8(x):
        return np.clip(x, -440.0, 440.0).astype(FP8)

    # fold LN affine into wq/w1 (exact: (xh*g+b)@W = xh@(g*W) + (b@W))
    g32, b32 = a32(ln_g), a32(ln_b)
    wq_e = a32(wq) * g32[:, None]
    bq_e = a32(bq) + b32 @ a32(wq)
    w1_e = a32(w1) * g32[:, None]
    b1_e = a32(b1) + b32 @ a32(w1)

    # wq8/wk8: [p, g, io, kc, ic, c] = S*W[256kc+128ic+p, 256g+64h'+32io+j], c=32h'+j
    def pack_qk(W, S):
        A = (a32(W) * S).reshape(4, 2, 128, 4, 4, 2, 32)  # [kc, ic, p, g, h', io, j]
        return q8(f(A.transpose(2, 3, 5, 0, 1, 4, 6).reshape(128, 4, 2, 4, 2, 128)))

    wq8 = pack_qk(wq_e, SQ)
    wk8 = pack_qk(wk, SK)
    # wv8: [p, g, kc, ic, j] = SV*wv[256kc+128ic+p, 256g+j]
    A = (a32(wv) * SV).reshape(4, 2, 128, 4, 256)          # [kc, ic, p, g, j]
    wv8 = q8(f(A.transpose(2, 3, 0, 1, 4)))
    # wo8: [p, g8, ic, m] = SO*wo[64*(2g8+ic)+p, m]
    A = (a32(wo) * SO).reshape(8, 2, 64, 1024)             # [g8, ic, p, m]
    wo8 = q8(f(A.transpose(2, 0, 1, 3)))
    # w1b: [hg, p, kc, hcol] ; w2b: [hm, p, m]
    A = w1_e.reshape(8, 128, 8, 512)                       # [kc, p, hg, hcol]
    w1bh = f(A.transpose(2, 1, 0, 3)).astype(BF)
    w2bh = f(a32(w2).reshape(32, 128, 1024)).astype(BF)

    # biasf [128, 48]
    biasf = np.zeros((128, 48), np.float32)
    bq_s = (SQ * bq_e).reshape(4, 4, 2, 32)                # [g, h', io, j]
    bk_s = (SK * a32(bk)).reshape(4, 4, 2, 32)
    for g in range(4):
        for io in range(2):
            biasf[:, 2 * g + io] = bq_s[g, :, io, :].reshape(128)
            biasf[:, 8 + 2 * g + io] = bk_s[g, :, io, :].reshape(128)
    biasf[:, 16:48] = b1_e.reshape(32, 128).T

    bvb = q8(np.tile((SV * a32(bv)).reshape(4, 1, 256), (1, 2, 1)).reshape(2048))
    bob = a32(bo).astype(BF)
    b2b = a32(b2).astype(BF)

    B = tgt.shape[0]
    emb8_by_b = {}
    for b in range(B):
        # emb8[p, kc, ic, n] = fp8(emb[n, 256kc+128ic+p])
        E = a32(emb_motion[b]).T.reshape(4, 2, 128, NK)    # [kc, ic, p, n]
        emb8_by_b[b] = q8(f(E.transpose(2, 0, 1, 3)))

    in_maps = []
    for c in range(8):
        b, h = divmod(c, 2)
        in_maps.append({
            "tgt": f(a32(tgt[b, h * NQ:(h + 1) * NQ])),
            "emb8": emb8_by_b[b],
            "wq8": wq8, "wk8": wk8, "wv8": wv8, "wo8": wo8,
            "w1b": w1bh, "w2b": w2bh,
            "biasf": biasf, "bvb": bvb, "bob": bob, "b2b": b2b,
        })
    r = run_bass_kernel_spmd(nc, in_maps, list(range(8)))
    res = np.empty((B, 1024, DIM), np.float32)
    for c in range(8):
        b, h = divmod(c, 2)
        res[b, h * NQ:(h + 1) * NQ] = r.results[c]["out"]
    return res


# revision 22
# speedup vs baseline: 1.4681x; 1.0049x over previous
import sys

sys.path.insert(0, "/opt/trn_rl_repo")
import numpy as np

DIM = 1024
HEADS = 16
HD = 64
HID = 4096
EPS = 1e-5
NQ = 512          # queries per core
NK = 2048
P = 128

SQ = 64.0         # wq scale (fp8 range)
SK = 64.0         # wk scale
SV = 64.0         # wv scale
SO = 64.0         # wo scale
ESC = 0.125 / (SQ * SK)   # exp scale: scores/temp with fp8 scales folded out
SCH_A = 12102203.161561486 * ESC   # 2^23/ln2, folded with ESC
SCH_B = 1064986823.0               # 127*2^23 - 366393 (min-RMS bias)
OSC = 1.0 / (SV * SO)     # out-proj descale (ctx8 = SV*ctx_normed, wo8 = SO*wo)

_CACHE = {}


def _build():
    import concourse.bacc as bacc
    import concourse.tile as tile
    from concourse import mybir
    from concourse.masks import make_identity
    from contextlib import ExitStack

    F32 = mybir.dt.float32
    BF16 = mybir.dt.bfloat16
    F8 = mybir.dt.float8e4
    AF = mybir.ActivationFunctionType
    DR = mybir.MatmulPerfMode.DoubleRow

    nc = bacc.Bacc(None, target_bir_lowering=False, debug=False)

    tgt = nc.declare_dram_parameter("tgt", [NQ, DIM], F32, isOutput=False)
    # emb8[p, kc, ic, n] = fp8(emb[n, 256*kc + 128*ic + p])
    emb8 = nc.declare_dram_parameter("emb8", [P, 4, 2, NK], F8, isOutput=False)
    # wq8[p, g, io, kc, ic, c(=32h'+j)] = SQ*wq_e[256kc+128ic+p, 256g+64h'+32io+j]
    wq8 = nc.declare_dram_parameter("wq8", [P, 4, 2, 4, 2, P], F8, isOutput=False)
    wk8 = nc.declare_dram_parameter("wk8", [P, 4, 2, 4, 2, P], F8, isOutput=False)
    # wv8[p, g, kc, ic, j(=64hq+d)] = SV*wv[256kc+128ic+p, 256g+j]
    wv8 = nc.declare_dram_parameter("wv8", [P, 4, 4, 2, 256], F8, isOutput=False)
    # wo8[p(0:64), g8, ic, m] = SO*wo[64*(2*g8+ic)+p, m]
    wo8 = nc.declare_dram_parameter("wo8", [64, 8, 2, DIM], F8, isOutput=False)
    # w1b[hg][p][kc, hcol] = bf16(w1_e[128kc+p, 512hg+hcol])
    w1b = nc.declare_dram_parameter("w1b", [8, P, 8, 512], BF16, isOutput=False)
    # w2b[hm][p][m] = bf16(w2[128hm+p, m])
    w2b = nc.declare_dram_parameter("w2b", [HID // P, P, DIM], BF16, isOutput=False)
    # biasf: [128, 48]: cols 0:8 SQ*bq_e by (g,io), 8:16 SK*bk, 16:48 b1_e by hm
    biasf = nc.declare_dram_parameter("biasf", [P, 48], F32, isOutput=False)
    bvb = nc.declare_dram_parameter("bvb", [2 * DIM], F8, isOutput=False)  # SV*bv, dup x2
    bob = nc.declare_dram_parameter("bob", [DIM], BF16, isOutput=False)   # bo
    b2b = nc.declare_dram_parameter("b2b", [DIM], BF16, isOutput=False)   # b2
    out = nc.declare_dram_parameter("out", [NQ, DIM], F32, isOutput=True)

    def bcast_ap(vec, n):
        import concourse.bass as bass
        return bass.AP(tensor=vec.tensor, offset=vec.offset, ap=[[0, P], [1, n]])

    with tile.TileContext(nc) as tc, ExitStack() as S:
        const = S.enter_context(tc.tile_pool(name="const", bufs=1))

        identf = const.tile([P, P], F32)
        make_identity(nc, identf)
        identb = const.tile([P, P], BF16)
        nc.scalar.activation(identb[:], identf[:], AF.Copy)
        eps_t = const.tile([P, 1], F32)
        nc.vector.memset(eps_t[:], EPS)

        bp = const.tile([P, 48], F32)
        bvb_b = const.tile([P, 2 * DIM], F8)
        bob_b = const.tile([P, DIM], BF16)
        b2b_b = const.tile([P, DIM], BF16)

        # ---- persistent SBUF ----
        perm = S.enter_context(tc.tile_pool(name="perm", bufs=1))
        eT8 = perm.tile([P, 4, 2, NK], F8)          # emb, feature-major fp8
        wk_sb = perm.tile([P, 4, 2, 4, 2, P], F8)
        tgt_t = [perm.tile([P, DIM], F32, name=f"tgt{t}") for t in range(4)]
        nc.sync.dma_start(out=wk_sb[:, 0], in_=wk8[:, 0])
        for kc in range(4):
            nc.sync.dma_start(out=eT8[:, kc, :, :], in_=emb8[:, kc, :, :])
        for g in range(1, 4):
            nc.sync.dma_start(out=wk_sb[:, g], in_=wk8[:, g])
        for t in range(4):
            nc.sync.dma_start(out=tgt_t[t][:], in_=tgt[t * P:(t + 1) * P, :])
        nc.sync.dma_start(out=bp[:], in_=biasf[:, :])
        wv_sb = perm.tile([P, 4, 4, 2, 256], F8)
        nc.sync.dma_start(out=wv_sb[:], in_=wv8[:, :, :, :, :])
        wo_sb = perm.tile([64, 8, 2, DIM], F8)
        nc.sync.dma_start(out=wo_sb[:], in_=wo8[:, :, :, :])
        nc.sync.dma_start(out=bvb_b[:], in_=bcast_ap(bvb[:], 2 * DIM))
        nc.sync.dma_start(out=bob_b[:], in_=bcast_ap(bob[:], DIM))
        nc.sync.dma_start(out=b2b_b[:], in_=bcast_ap(b2b[:], DIM))

        K8g = [perm.tile([P, 2, NK], F8, name=f"K8_{g}") for g in range(4)]
        Q8g = [perm.tile([P, 2, NQ], F8, name=f"Q8_{g}") for g in range(4)]
        # head 3 of each group sits at partition base 96, which matmul APs
        # can't address -> DMA-shift its 32 partitions down to base 0
        K8h3 = [perm.tile([32, 2, NK], F8, name=f"K8h3_{g}") for g in range(4)]
        Q8h3 = [perm.tile([32, 2, NQ], F8, name=f"Q8h3_{g}") for g in range(4)]
        lnT8 = [perm.tile([P, 2, NQ], F8, name=f"lnT8_{k}") for k in range(4)]
        ctx8 = [perm.tile([64, 2, NQ], F8, name=f"ctx8_{g8}") for g8 in range(8)]
        tgt2 = [perm.tile([P, DIM], F32, name=f"tgt2_{t}") for t in range(4)]
        ln2T = [[perm.tile([P, P], BF16, name=f"ln2T_{k}_{t}") for t in range(4)]
                for k in range(8)]
        h1T = [perm.tile([P, NQ], BF16, name=f"h1T_{m}") for m in range(HID // P)]

        def layer_norm_rows(x, y, pool, nm):
            # y = (x - mean)/std rowwise over 1024, y may be bf16
            st = pool.tile([P, 2, nc.vector.BN_STATS_DIM], F32, name=f"st{nm}", tag="st")
            for sg in range(2):
                nc.vector.bn_stats(out=st[:, sg, :], in_=x[:, sg * 512:(sg + 1) * 512])
            mv = pool.tile([P, nc.vector.BN_AGGR_DIM], F32, name=f"mv{nm}", tag="mv")
            nc.vector.bn_aggr(out=mv[:], in_=st[:])
            rstd = pool.tile([P, 1], F32, name=f"rstd{nm}", tag="rstd")
            nc.scalar.activation(out=rstd[:], in_=mv[:, 1:2], func=AF.Sqrt,
                                 bias=eps_t[:], scale=1.0)
            nc.vector.reciprocal(out=rstd[:], in_=rstd[:])
            nb = pool.tile([P, 1], F32, name=f"nb{nm}", tag="nb")
            nc.vector.tensor_scalar(out=nb[:], in0=mv[:, 0:1], scalar1=rstd[:],
                                    scalar2=-1.0, op0=mybir.AluOpType.mult,
                                    op1=mybir.AluOpType.mult)
            nc.scalar.activation(out=y[:], in_=x[:], func=AF.Identity,
                                 bias=nb[:], scale=rstd[:])

        # ---------- Phase 1 + 2: projections and attention, software-pipelined ----------
        kq_cm = tc.tile_pool(name="kq_ps", bufs=2, space="PSUM")
        kq_ps = kq_cm.__enter__()

        ln1_cm = tc.tile_pool(name="ln1", bufs=1)
        ln1 = ln1_cm.__enter__()

        wq_cm = tc.tile_pool(name="wqp", bufs=1)
        wqp = wq_cm.__enter__()
        wq_sb = wqp.tile([P, 4, 2, 4, 2, P], F8)
        nc.sync.dma_start(out=wq_sb[:], in_=wq8[:, :, :, :, :, :])

        def emit_kproj(g, ios=(0, 1)):
            # kproj group g -> K8g[g] (fp8 DoubleRow, contraction 256)
            # group 0 converts on Act (pre-phase); later groups on DVE
            for io in ios:
                for nck in range(4):
                    ps = kq_ps.tile([P, NQ], F32, name="kps", tag="kq")
                    for kc in range(4):
                        nc.tensor.matmul(
                            ps[:], wk_sb[:, g, io, kc, :, :],
                            eT8[:, kc, :, nck * NQ:(nck + 1) * NQ],
                            start=(kc == 0), stop=(kc == 3), perf_mode=DR)
                    if g == 0:
                        nc.scalar.activation(
                            K8g[g][:, io, nck * NQ:(nck + 1) * NQ], ps[:],
                            AF.Identity, bias=bp[:, 8 + 2 * g + io:9 + 2 * g + io])
                    else:
                        nc.vector.tensor_scalar_add(
                            K8g[g][:, io, nck * NQ:(nck + 1) * NQ], ps[:],
                            bp[:, 8 + 2 * g + io:9 + 2 * g + io])
            if ios[-1] == 1:
                nc.sync.dma_start(out=K8h3[g][:], in_=K8g[g][96:128, :, :])

        emit_kproj(0)
        with tc.tile_pool(name="lnw", bufs=4) as lnw, \
             tc.tile_pool(name="tp_ps", bufs=4, space="PSUM") as tp_ps:
            # LN1 rows (bf16) on DVE while kproj g0 runs on PE
            ln1r = [ln1.tile([P, DIM], BF16, name=f"ln1r{t}") for t in range(4)]
            for t in range(4):
                layer_norm_rows(tgt_t[t], ln1r[t], lnw, f"a{t}")
            for t in range(4):
                for kc in range(4):
                    for ic in range(2):
                        pt = tp_ps.tile([P, P], BF16, name="pt", tag="tp")
                        f0 = 256 * kc + 128 * ic
                        nc.tensor.transpose(pt[:], ln1r[t][:, f0:f0 + P], identb[:])
                        nc.vector.tensor_copy(lnT8[kc][:, ic, t * P:(t + 1) * P], pt[:])
            # qproj -> Q8 (converts on Act)
            for g in range(4):
                for io in range(2):
                    ps = kq_ps.tile([P, NQ], F32, name="qps", tag="kq")
                    for kc in range(4):
                        nc.tensor.matmul(ps[:], wq_sb[:, g, io, kc, :, :], lnT8[kc][:],
                                         start=(kc == 0), stop=(kc == 3), perf_mode=DR)
                    nc.scalar.activation(Q8g[g][:, io, :], ps[:], AF.Identity,
                                         bias=bp[:, 2 * g + io:2 * g + io + 1])
                nc.sync.dma_start(out=Q8h3[g][:], in_=Q8g[g][96:128, :, :])
        wq_cm.__exit__(None, None, None)
        ln1_cm.__exit__(None, None, None)

        # ---------- attention (fp8 DoubleRow), kproj/vproj g>=1 interleaved ----------
        with tc.tile_pool(name="v8p", bufs=2) as v8p, \
             tc.tile_pool(name="exp8", bufs=3) as exp8, \
             tc.tile_pool(name="sm", bufs=2) as sm, \
             tc.tile_pool(name="scp", bufs=2, space="PSUM") as scp, \
             tc.tile_pool(name="cxp", bufs=2, space="PSUM") as cxp:

            v8 = {}

            def emit_vproj(g, ts):
                # v8[g][t]: [128, 2(ic=kvt parity), 4(hq), 68] fp8; col 64 = ones
                if g not in v8:
                    v8[g] = [v8p.tile([P, 2, 4, 68], F8, name=f"v8_{g}_{t}", tag=f"v{t}")
                             for t in range(8)]
                for t in ts:
                    ps = kq_ps.tile([P, NQ], F32, name="vp", tag="kq")
                    for half in range(2):   # kvt = 2t + half
                        kvt = 2 * t + half
                        for kc in range(4):
                            nc.tensor.matmul(
                                ps[:, half * 256:(half + 1) * 256],
                                eT8[:, kc, :, kvt * P:(kvt + 1) * P],
                                wv_sb[:, g, kc, :, :],
                                start=(kc == 0), stop=(kc == 3), perf_mode=DR)
                    nc.vector.tensor_tensor(
                        out=v8[g][t][:, :, :, 0:64],
                        in0=ps[:].rearrange("p (i h d) -> p i h d", i=2, h=4),
                        in1=bvb_b[:, 512 * g:512 * (g + 1)].rearrange(
                            "p (i h d) -> p i h d", i=2, h=4),
                        op=mybir.AluOpType.add)
                    nc.gpsimd.memset(v8[g][t][:, :, :, 64:65], 1.0)

            emit_vproj(0, range(8))
            for g in range(4):
                for h in range(4):
                    head = 4 * g + h
                    g8, ih = divmod(head, 2)
                    cps = cxp.tile([P, NQ], F32, name="cps", tag="cps")
                    if h < 3:
                        Ksrc, Qsrc, pb = K8g[g], Q8g[g], 32 * h
                    else:
                        Ksrc, Qsrc, pb = K8h3[g], Q8h3[g], 0
                    for t in range(8):
                        sc = scp.tile([P, 2 * NQ], F32, name="sc", tag="sc")
                        for half in range(2):
                            kvt = 2 * t + half
                            nc.tensor.matmul(
                                sc[:, half * NQ:(half + 1) * NQ],
                                Ksrc[pb:pb + 32, :, kvt * P:(kvt + 1) * P],
                                Qsrc[pb:pb + 32, :, :],
                                start=True, stop=True, perf_mode=DR)
                        ex = exp8.tile([P, 2 * NQ], F8, name="ex", tag="ex")
                        nc.scalar.activation(ex[:], sc[:], AF.Exp, scale=ESC)
                        nc.tensor.matmul(
                            cps[0:65, :], v8[g][t][:, :, h, 0:65],
                            ex[:].rearrange("p (i n) -> p i n", i=2),
                            start=(t == 0), stop=(t == 7), perf_mode=DR)
                    # interleave next quarter's vproj/kproj behind this quarter
                    if g < 3:
                        if h == 0:
                            emit_vproj(g + 1, range(0, 4))
                        elif h == 1:
                            emit_vproj(g + 1, range(4, 8))
                        elif h == 2:
                            emit_kproj(g + 1, (0,))
                        else:
                            emit_kproj(g + 1, (1,))
                    # softmax tail: normalize by denominator (row 64)
                    rl = sm.tile([P, NQ], F32, name="rl", tag="rl")
                    nc.vector.reciprocal(out=rl[64:65, :], in_=cps[64:65, :])
                    rl0 = sm.tile([1, NQ], F32, name="rl0", tag="rl0")
                    nc.sync.dma_start(out=rl0[0:1, :], in_=rl[64:65, :])
                    bcs = sm.tile([64, NQ], F32, name="bcs", tag="bcs")
                    nc.gpsimd.partition_broadcast(bcs[:], rl0[0:1, :], channels=64)
                    nc.vector.tensor_tensor(
                        out=ctx8[g8][0:64, ih, :], in0=cps[0:64, :], in1=bcs[:],
                        op=mybir.AluOpType.mult)

        kq_cm.__exit__(None, None, None)

        # ---------- Phase 3: out-proj + residual -> tgt2; LN2 -> ln2T ----------
        ln2_cm = tc.tile_pool(name="ln2", bufs=1)
        ln2 = ln2_cm.__enter__()
        ln2r = [ln2.tile([P, DIM], BF16, name=f"ln2r{t}") for t in range(4)]
        with tc.tile_pool(name="ln2w", bufs=4) as ln2w, \
             tc.tile_pool(name="o_ps", bufs=2, space="PSUM") as o_ps, \
             tc.tile_pool(name="l2_ps", bufs=4, space="PSUM") as l2_ps:
            for t in range(4):
                # fold +bo into residual before the STT add
                nc.gpsimd.tensor_tensor(out=tgt_t[t][:], in0=tgt_t[t][:],
                                        in1=bob_b[:], op=mybir.AluOpType.add)
                ps = o_ps.tile([P, DIM], F32, name="ops", tag="op")
                for g8 in range(8):
                    for mh in range(2):
                        nc.tensor.matmul(ps[:, mh * 512:(mh + 1) * 512],
                                         ctx8[g8][:, :, t * P:(t + 1) * P],
                                         wo_sb[:, g8, :, mh * 512:(mh + 1) * 512],
                                         start=(g8 == 0), stop=(g8 == 7), perf_mode=DR)
                nc.vector.scalar_tensor_tensor(
                    out=tgt2[t][:], in0=ps[:], scalar=OSC, in1=tgt_t[t][:],
                    op0=mybir.AluOpType.mult, op1=mybir.AluOpType.add)
                layer_norm_rows(tgt2[t], ln2r[t], ln2w, f"b{t}")
                # b2 folded into tgt2 AFTER stats are taken (fc2 residual)
                nc.gpsimd.tensor_tensor(out=tgt2[t][:], in0=tgt2[t][:],
                                        in1=b2b_b[:], op=mybir.AluOpType.add)
                for k in range(8):
                    pt = l2_ps.tile([P, P], BF16, name="l2pt", tag="l2tp")
                    nc.tensor.transpose(pt[:], ln2r[t][:, k * P:(k + 1) * P], identb[:])
                    nc.scalar.activation(ln2T[k][t][:], pt[:], AF.Copy)
        ln2_cm.__exit__(None, None, None)

        # ---------- Phase 4: fc1 (bf16) + gelu ----------
        with tc.tile_pool(name="w1s", bufs=2) as w1s, \
             tc.tile_pool(name="f1_ps", bufs=2, space="PSUM") as f1_ps:
            for hg in range(8):
                wt = w1s.tile([P, 8, 512], BF16, name="w1t", tag="w1")
                nc.sync.dma_start(out=wt[:], in_=w1b[hg])
                for sub in range(2):
                    pss = [f1_ps.tile([P, NQ], F32, name=f"f1p{j}", tag=f"f1_{j}")
                           for j in range(2)]
                    for qt in range(4):
                        for kc in range(8):
                            for j in range(2):
                                hc = 2 * sub + j
                                nc.tensor.matmul(
                                    pss[j][:, qt * P:(qt + 1) * P],
                                    wt[:, kc, hc * P:(hc + 1) * P],
                                    ln2T[kc][qt][:], start=(kc == 0), stop=(kc == 7))
                    for j in range(2):
                        hm = hg * 4 + 2 * sub + j
                        nc.scalar.activation(h1T[hm][:], pss[j][:], AF.Gelu,
                                             bias=bp[:, 16 + hm:17 + hm])

        # ---------- Phase 5: fc2 (bf16) + residual + store ----------
        with tc.tile_pool(name="w2s", bufs=4) as w2s, \
             tc.tile_pool(name="f2_ps", bufs=1, space="PSUM") as f2_ps:
            pss = [f2_ps.tile([P, DIM], F32, name=f"f2p{t}", tag=f"f2_{t}")
                   for t in range(4)]
            for hm in range(HID // P):
                wt = w2s.tile([P, DIM], BF16, name="w2t", tag="w2")
                nc.sync.dma_start(out=wt[:], in_=w2b[hm])
                for t in range(4):
                    for mh in range(2):
                        nc.tensor.matmul(pss[t][:, mh * 512:(mh + 1) * 512],
                                         h1T[hm][:, t * P:(t + 1) * P],
                                         wt[:, mh * 512:(mh + 1) * 512],
                                         start=(hm == 0), stop=(hm == HID // P - 1))
            for t in range(4):
                nc.vector.tensor_tensor(out=tgt_t[t][:], in0=pss[t][:],
                                        in1=tgt2[t][:], op=mybir.AluOpType.add)
                nc.sync.dma_start(out=out[t * P:(t + 1) * P, :], in_=tgt_t[t][:])

    nc.compile()
    return nc


def _get_nc():
    if "nc" not in _CACHE:
        _CACHE["nc"] = _build()
    return _CACHE["nc"]


def kernel(tgt, emb_motion, ln_g, ln_b, wq, bq, wk, bk, wv, bv, wo, bo, w1, b1, w2, b2):
    import ml_dtypes
    from concourse.bass_utils import run_bass_kernel_spmd

    nc = _get_nc()
    f = np.ascontiguousarray
    a32 = lambda x: np.asarray(x, np.float32)
    FP8 = ml_dtypes.float8_e4m3
    BF = ml_dtypes.bfloat16

    def q8(x):
        return np.clip(x, -440.0, 440.0).astype(FP8)

    # fold LN affine into wq/w1 (exact: (xh*g+b)@W = xh@(g*W) + (b@W))
    g32, b32 = a32(ln_g), a32(ln_b)
    wq_e = a32(wq) * g32[:, None]
    bq_e = a32(bq) + b32 @ a32(wq)
    w1_e = a32(w1) * g32[:, None]
    b1_e = a32(b1) + b32 @ a32(w1)

    # wq8/wk8: [p, g, io, kc, ic, c] = S*W[256kc+128ic+p, 256g+64h'+32io+j], c=32h'+j
    def pack_qk(W, S):
        A = (a32(W) * S).reshape(4, 2, 128, 4, 4, 2, 32)  # [kc, ic, p, g, h', io, j]
        return q8(f(A.transpose(2, 3, 5, 0, 1, 4, 6).reshape(128, 4, 2, 4, 2, 128)))

    wq8 = pack_qk(wq_e, SQ)
    wk8 = pack_qk(wk, SK)
    # wv8: [p, g, kc, ic, j] = SV*wv[256kc+128ic+p, 256g+j]
    A = (a32(wv) * SV).reshape(4, 2, 128, 4, 256)          # [kc, ic, p, g, j]
    wv8 = q8(f(A.transpose(2, 3, 0, 1, 4)))
    # wo8: [p, g8, ic, m] = SO*wo[64*(2g8+ic)+p, m]
    A = (a32(wo) * SO).reshape(8, 2, 64, 1024)             # [g8, ic, p, m]
    wo8 = q8(f(A.transpose(2, 0, 1, 3)))
    # w1b: [hg, p, kc, hcol] ; w2b: [hm, p, m]
    A = w1_e.reshape(8, 128, 8, 512)                       # [kc, p, hg, hcol]
    w1bh = f(A.transpose(2, 1, 0, 3)).astype(BF)
    w2bh = f(a32(w2).reshape(32, 128, 1024)).astype(BF)

    # biasf [128, 48]
    biasf = np.zeros((128, 48), np.float32)
    bq_s = (SQ * bq_e).reshape(4, 4, 2, 32)                # [g, h', io, j]
    bk_s = (SK * a32(bk)).reshape(4, 4, 2, 32)
    for g in range(4):
        for io in range(2):
            biasf[:, 2 * g + io] = bq_s[g, :, io, :].reshape(128)
            biasf[:, 8 + 2 * g + io] = bk_s[g, :, io, :].reshape(128)
    biasf[:, 16:48] = b1_e.reshape(32, 128).T

    bvb = q8(np.tile((SV * a32(bv)).reshape(4, 1, 256), (1, 2, 1)).reshape(2048))
    bob = a32(bo).astype(BF)
    b2b = a32(b2).astype(BF)

    B = tgt.shape[0]
    emb8_by_b = {}
    for b in range(B):
        # emb8[p, kc, ic, n] = fp8(emb[n, 256kc+128ic+p])
        E = a32(emb_motion[b]).T.reshape(4, 2, 128, NK)    # [kc, ic, p, n]
        emb8_by_b[b] = q8(f(E.transpose(2, 0, 1, 3)))

    in_maps = []
    for c in range(8):
        b, h = divmod(c, 2)
        in_maps.append({
            "tgt": f(a32(tgt[b, h * NQ:(h + 1) * NQ])),
            "emb8": emb8_by_b[b],
            "wq8": wq8, "wk8": wk8, "wv8": wv8, "wo8": wo8,
            "w1b": w1bh, "w2b": w2bh,
            "biasf": biasf, "bvb": bvb, "bob": bob, "b2b": b2b,
        })
    r = run_bass_kernel_spmd(nc, in_maps, list(range(8)))
    res = np.empty((B, 1024, DIM), np.float32)
    for c in range(8):
        b, h = divmod(c, 2)
        res[b, h * NQ:(h + 1) * NQ] = r.results[c]["out"]
    return res
